# revision 1
# baseline (speedup 1.0000x reference)
"""BitLinear (x @ ternary_kernel + bias) on 8 Trainium2 NeuronCores.

Strategy: data-parallel over the batch dim (8 batches -> 8 cores). Each core
computes out_b = x_b @ W + bias for x_b [2048, 4096], W [4096, 4096].

The matmul runs in bf16 (weights are ternary {-1,0,1} -> exact in bf16; x
rounds to bf16, products are exact signed copies, accumulation is fp32 in
PSUM), giving ~1e-3 relative error vs the fp32 reference.

Host-side prep (free wrt device time): cast to bf16 and pre-transpose x so
both matmul operands have the contraction dim on SBUF partitions, in the
[p, k_outer, free] tiled DRAM layout that matmul_tile_kernel expects.
"""

import numpy as np
import ml_dtypes

import concourse.bacc as bacc
import concourse.mybir as mybir
import concourse.tile as tile
from concourse.bass_utils import run_bass_kernel_spmd
from concourse.kernels.tile_matmul import matmul_tile_kernel

B, T, D, U = 8, 2048, 4096, 4096
P = 128
KO = D // P  # 32 k-outer tiles
MO = T // P  # 16 m-outer tiles
N_CORES = 8

_BF16 = ml_dtypes.bfloat16

_cached_nc = None


def _build_program():
    """One SPMD Bass program: mxn[p,mo,U] = kxm[p,ko,T].T @ kxn[p,ko,U]."""
    nc = bacc.Bacc("TRN2", target_bir_lowering=False, debug=False,
                   num_devices=N_CORES)
    kxm = nc.dram_tensor("kxm", [P, KO, T], mybir.dt.bfloat16,
                         kind="ExternalInput").ap()
    kxn = nc.dram_tensor("kxn", [P, KO, U], mybir.dt.bfloat16,
                         kind="ExternalInput").ap()
    mxn = nc.dram_tensor("mxn", [P, MO, U], mybir.dt.float32,
                         kind="ExternalOutput").ap()
    with tile.TileContext(nc) as tc:
        matmul_tile_kernel(tc, kxm, kxn, mxn)
    nc.compile()
    return nc


def _get_program():
    global _cached_nc
    if _cached_nc is None:
        _cached_nc = _build_program()
    return _cached_nc


def make_in_maps(x, kernel):
    """Host-side shard + layout prep. Returns per-core input maps."""
    x = np.asarray(x)
    w = np.asarray(kernel)
    # W[d, u] -> [p, ko, u] with d = ko*128 + p; shared by all cores.
    w_t = np.ascontiguousarray(
        w.astype(_BF16).reshape(KO, P, U).transpose(1, 0, 2))
    in_maps = []
    for b in range(B):
        # x_b[m, d] -> xT tiled [p, ko, m] with d = ko*128 + p.
        xb = np.ascontiguousarray(
            x[b].astype(_BF16).reshape(T, KO, P).transpose(2, 1, 0))
        in_maps.append({"kxm": xb, "kxn": w_t})
    return in_maps


def assemble_output(results, bias):
    """Per-core mxn [p, mo, U] -> full [B, T, U] fp32 (+bias)."""
    bias = np.asarray(bias, dtype=np.float32)
    out = np.empty((B, T, U), dtype=np.float32)
    for b in range(B):
        mxn = results[b]["mxn"]  # [P, MO, U], m = mo*128 + p
        out[b] = mxn.transpose(1, 0, 2).reshape(T, U)
    if np.any(bias):
        out += bias[None, None, :]
    return out


def kernel(x, kernel, bias):
    nc = _get_program()
    in_maps = make_in_maps(x, kernel)
    res = run_bass_kernel_spmd(nc, in_maps, core_ids=list(range(N_CORES)))
    return assemble_output(res.results, bias)


# revision 12
# speedup vs baseline: 4.0595x; 4.0595x over previous
"""BitLinear (x @ ternary_kernel + bias) on 8 Trainium2 NeuronCores.

Strategy: data-parallel over the batch dim (8 batches -> 8 cores). Each core
computes out_b = x_b @ W for x_b [2048, 4096], W [4096, 4096], bf16 matmul
with fp32 PSUM accumulation (~1.7e-3 rel err vs fp32 reference; W is ternary
so it is exact in bf16).

Per-core kernel: x_b^T stays fully resident in SBUF (16 MiB as 16 m-tiles of
[128k x 32ko x 128m]); W streams as 8 column chunks of [128k x 32ko x 512u]
(4 MiB each, double-buffered), each reused across all 16 m-tiles so the PE
gets ~109us of dense matmuls per 11us prefetch and never goes cold. PSUM
tiles [128m x 512u] accumulate 32 matmuls over K, evicted via DVE copy and
DMA'd straight to the natural [2048, 4096] fp32 output layout.

Host-side prep (free wrt device time): bf16 cast + retile so every DMA is
fully contiguous in DRAM.
"""

import numpy as np
import ml_dtypes

import concourse.bacc as bacc
import concourse.mybir as mybir
import concourse.tile as tile
from concourse.bass_utils import run_bass_kernel_spmd

B, T, D, U = 8, 2048, 4096, 4096
P = 128
KO = D // P      # 32 k-tiles of 128
MO = T // P      # 16 m-tiles of 128
NF = 512         # psum free dim (one bank)
NO = U // NF     # 8 n-chunks
N_CORES = 8

_BF16 = ml_dtypes.bfloat16

_cached_nc = None


def _build_program():
    nc = bacc.Bacc("TRN2", target_bir_lowering=False, debug=False,
                   num_devices=N_CORES)
    bf16 = mybir.dt.bfloat16
    f32 = mybir.dt.float32
    xt_d = nc.dram_tensor("xt", [MO, P, KO, P], bf16,
                          kind="ExternalInput").ap()
    w_d = nc.dram_tensor("w", [NO, P, KO, NF], bf16,
                         kind="ExternalInput").ap()
    out_d = nc.dram_tensor("out", [T, U], f32, kind="ExternalOutput").ap()

    with tile.TileContext(nc) as tc:
        KQ = KO // 4  # 8 k-tiles per W quarter-tile
        with (
            tc.tile_pool(name="xpool", bufs=MO) as xpool,
            tc.tile_pool(name="wpool", bufs=8) as wpool,
            tc.tile_pool(name="opool", bufs=4) as opool,
            tc.tile_pool(name="psum", bufs=8, space="PSUM") as psum_pool,
        ):
            # Emission order matters: only xt[0] + the first W quarter
            # (1 MiB) gate the first matmul; the other x tiles and W
            # quarters stream in behind and hide under compute.
            from concourse.tile_rust import add_dep_helper

            def load_w_chunk(no):
                qs, insts = [], []
                for q in range(4):
                    wq = wpool.tile([P, KQ, NF], bf16, tag="w")
                    di = nc.sync.dma_start(
                        out=wq[:],
                        in_=w_d[no, :, q * KQ:(q + 1) * KQ, :])
                    qs.append(wq)
                    insts.append(di)
                return qs, insts

            xtiles = []
            xt = xpool.tile([P, KO, P], bf16, tag="x")
            nc.sync.dma_start(out=xt[:], in_=xt_d[0])
            xtiles.append(xt)
            wt0, w0_insts = load_w_chunk(0)
            for mo in range(1, MO):
                xt = xpool.tile([P, KO, P], bf16, tag="x")
                di = nc.sync.dma_start(out=xt[:], in_=xt_d[mo])
                # Keep these 15 loads out of the SDMA rings until the
                # gating first W quarter has landed, so it gets the HBM
                # bandwidth during the startup window.
                add_dep_helper(di.ins if hasattr(di, "ins") else di,
                               w0_insts[0].ins if hasattr(w0_insts[0], "ins")
                               else w0_insts[0],
                               reason="delay xt prefetch past first W quarter")
                xtiles.append(xt)
            for no in range(NO):
                wt = wt0 if no == 0 else load_w_chunk(no)[0]
                for mo in range(MO):
                    ps = psum_pool.tile([P, NF], f32)
                    for ko in range(KO):
                        nc.tensor.matmul(ps[:], lhsT=xtiles[mo][:, ko, :],
                                         rhs=wt[ko // KQ][:, ko % KQ, :],
                                         start=(ko == 0), stop=(ko == KO - 1))
                    ob = opool.tile([P, NF], f32)
                    nc.vector.tensor_copy(out=ob[:], in_=ps[:])
                    # scalar HWDGE queue: keeps output stores off the sync
                    # queue that feeds the critical x/W prefetches
                    nc.scalar.dma_start(
                        out=out_d[mo * P:(mo + 1) * P, no * NF:(no + 1) * NF],
                        in_=ob[:])
    nc.compile()
    return nc


def _get_program():
    global _cached_nc
    if _cached_nc is None:
        _cached_nc = _build_program()
    return _cached_nc


def make_in_maps(x, kernel):
    """Host-side shard + layout prep. Returns per-core input maps."""
    x = np.asarray(x)
    w = np.asarray(kernel)
    # w[no, p, ko, ni] = W[ko*128+p, no*512+ni]; shared by all cores.
    w_t = np.ascontiguousarray(
        w.astype(_BF16).reshape(KO, P, NO, NF).transpose(2, 1, 0, 3))
    in_maps = []
    for b in range(B):
        # xt[mo, p, ko, mi] = x[b, mo*128+mi, ko*128+p]
        xb = np.ascontiguousarray(
            x[b].astype(_BF16).reshape(MO, P, KO, P).transpose(0, 3, 2, 1))
        in_maps.append({"xt": xb, "w": w_t})
    return in_maps


def assemble_output(results, bias):
    bias = np.asarray(bias, dtype=np.float32)
    out = np.empty((B, T, U), dtype=np.float32)
    for b in range(B):
        out[b] = results[b]["out"]
    if np.any(bias):
        out += bias[None, None, :]
    return out


def kernel(x, kernel, bias):
    nc = _get_program()
    in_maps = make_in_maps(x, kernel)
    last_err = None
    for attempt in range(3):
        try:
            res = run_bass_kernel_spmd(nc, in_maps,
                                       core_ids=list(range(N_CORES)))
            return assemble_output(res.results, bias)
        except Exception as e:  # transient device wedge (NRT_EXEC_UNIT_...)
            last_err = e
            try:
                import jax
                jax.clear_caches()
                jax.extend.backend.clear_backends()
            except Exception:
                pass
    raise last_err
